# revision 10
# baseline (speedup 1.0000x reference)
"""BitLinear (ternary-quantized linear) Trainium2 kernel.

out = x @ (gamma * ternary(weight)).T + bias, computed tensor-parallel over
8 NeuronCores: weight/bias sharded along out_features, x replicated.

Per-core device program (hybrid fp8-DoubleRow / bf16 matmul):
  1. Cast x (fp32) -> bf16 into DRAM scratch via SWDGE casting DMAs,
     throttled to stay a few m-tiles ahead of consumption.
  2. Quantize the weight shard to doubled ternary {-2,0,2} on the ACT
     engine: q' = sign(w - thr) + sign(w + thr) with thr = 0.5*gamma,
     equivalent to 2*clip(round(w/gamma), -1, 1) for all inputs; the factor
     2 is folded into the output scale (gamma/2, exact in fp32).
  3. PE-transpose q' into SBUF-resident [K-partition, k-subtile, N] weight
     tiles: the first KT8 k-subtiles in fp8e4 (exact for {-2,0,2}), the
     remaining KT16 subtiles in bf16. XBAR DMA-transpose x_bf16 tiles into
     [K-partition] layout; ACT-cast the first KT8 subtiles to fp8e4.
  4. Matmuls accumulate fp32 in PSUM: the fp8 k-range runs as DoubleRow
     matmuls (2 k-subtiles / 256 contraction rows per instruction, 2x PE
     throughput — measured streaming at the same ~219 ns per 512-wide
     matmul as bf16), the bf16 k-range as regular 128x128x512 matmuls.
     The fp8 path quantizes x to e4m3 (~2.7e-2 per-element rel RMS);
     keeping KT16 subtiles in bf16 scales the output error by
     ~sqrt(KT8/32), keeping it under the accuracy gate (measured
     1.64e-2 at KT8=16, 1.7-1.8e-2 expected at KT8=18, gate 2e-2).
  5. Drain: psum * (gamma/2) + bias on DVE, DMA out on the Scalar queue.

gamma = max(mean(|clip(w, -2, 2)|), 1e-4) is a global scalar over the full
weight; it is computed on host with the same jnp ops the module uses so the
quantization boundary matches bit-exactly, and enters the device kernel as a
[128, 4] scalar input tensor (threshold, -threshold, gamma).
"""

import numpy as np

import concourse.bass as bass
import concourse.mybir as mybir
import concourse.tile as tile
from concourse import bacc
from concourse.bass_utils import run_bass_kernel_spmd
from concourse.tile import add_dep_helper
from concourse.masks import make_identity

P = 128
B, S, D_IN, D_OUT = 4, 2048, 4096, 16384
M = B * S                 # 8192 tokens
K = D_IN                  # 4096 contraction
N_CORES = 8
NS = D_OUT // N_CORES     # 2048 out-features per core
KT = K // P               # 32 k-subtiles
MT = M // P               # 64 m-tiles
NBS = 512                 # psum bank free size (fp32)
NB = NS // NBS            # 4 psum n-blocks
QCH = 1024                # weight-quantize chunk free size

PA = 9                    # fp8 DoubleRow k-subtile PAIRS (2*PA subtiles fp8)
KT8 = 2 * PA              # k-subtiles on the fp8 path
KT16 = KT - KT8           # k-subtiles on the bf16 path

F32 = mybir.dt.float32
BF16 = mybir.dt.bfloat16
FP8 = mybir.dt.float8e4
DR = mybir.MatmulPerfMode.DoubleRow

_NC_CACHE = None
LAST_RESULTS = None


def _build_nc():
    nc = bacc.Bacc(None, target_bir_lowering=False, debug=False)

    x_in = nc.declare_dram_parameter("x", [M, K], F32, isOutput=False)
    w_in = nc.declare_dram_parameter("w", [NS, K], F32, isOutput=False)
    b_in = nc.declare_dram_parameter("bias", [P, NS], F32, isOutput=False)
    s_in = nc.declare_dram_parameter("scal", [P, 4], F32, isOutput=False)
    y_out = nc.declare_dram_parameter("out", [M, NS], F32, isOutput=True)

    CAST_AHEAD = 6
    RAMP_TILES = 16

    with tile.TileContext(nc) as tc:
        with (
            tc.tile_pool(name="const", bufs=1) as constp,
            tc.tile_pool(name="w_sb", bufs=4) as wsbp,
            tc.tile_pool(name="qab", bufs=8) as qabp,
            tc.tile_pool(name="xT", bufs=3) as xTp,
            tc.tile_pool(name="xT8", bufs=3) as xT8p,
            tc.tile_pool(name="osb", bufs=3) as osbp,
            tc.tile_pool(name="psum", bufs=8, space="PSUM") as psump,
            tc.tile_pool(name="dram", bufs=1, space="DRAM") as dramp,
        ):
            scal = constp.tile([P, 4], F32)
            nc.sync.dma_start(out=scal[:], in_=s_in[:])
            bias_sb = constp.tile([P, NS], F32)
            nc.sync.dma_start(out=bias_sb[:], in_=b_in[:])
            # full quantized-transposed weight shard, resident in SBUF:
            # fp8 range (DoubleRow path) + bf16 range
            wqT8 = constp.tile([P, KT8, NS], FP8, name="wqT8")
            wqT16 = constp.tile([P, KT16, NS], BF16, name="wqT16")

            # identity for PE transposes: emitted before the cast DMAs so it
            # is not queued behind them on the gpsimd queue
            ident = constp.tile([P, P], BF16)
            make_identity(nc, ident)

            # ---- x fp32 -> bf16 cast, DRAM->DRAM on SWDGE ----
            # Throttled below so the casts stay a few m-tiles ahead of
            # consumption instead of hogging HBM during the prologue.
            xhat = []
            cast_insts = []
            for j in range(MT):
                xh = dramp.tile([P, K], BF16, name=f"xhat_{j}")
                ci = nc.gpsimd.dma_start(out=xh[:], in_=x_in[j * P:(j + 1) * P, :])
                xhat.append(xh)
                cast_insts.append(ci)

            # ---- weight shard: quantize to doubled-ternary, transpose ----
            # q' = sign(w - thr) + sign(w + thr) in {-2, 0, 2}; the factor 2
            # is folded into the output scale (gamma/2). Signs run on the
            # otherwise-idle ACT engine, transposes on the PE, the add and
            # psum-evict on DVE. Subtiles < KT8 land in wqT8 as fp8e4
            # ({-2,0,2} is exact), the rest in wqT16 as bf16.
            for r in range(NS // P):
                for c in range(K // QCH):
                    w_sb = wsbp.tile([P, QCH], F32, tag="w_in")
                    # weight chunks ride the Scalar HWDGE queue (idle until
                    # outputs start) so the Sync queue only carries the x
                    # transposes: the Sync queue otherwise paces the whole
                    # quantize prologue (~190us of chunk loads)
                    nc.scalar.dma_start(
                        out=w_sb[:],
                        in_=w_in[r * P:(r + 1) * P, c * QCH:(c + 1) * QCH],
                    )
                    sa = qabp.tile([P, QCH], BF16, tag="q")
                    sb = qabp.tile([P, QCH], BF16, tag="q")
                    nc.scalar.sign(sa[:], w_sb[:], bias=scal[:, 1:2])  # -thr
                    nc.scalar.sign(sb[:], w_sb[:], bias=scal[:, 0:1])  # +thr
                    nc.vector.tensor_tensor(
                        sa[:], sa[:], sb[:], mybir.AluOpType.add
                    )
                    for kk in range(QCH // P):
                        po = c * (QCH // P) + kk
                        psA = psump.tile([P, P], BF16, tag="ps", name=f"tp_{r}_{po}")
                        nc.tensor.transpose(psA[:], sa[:, kk * P:(kk + 1) * P], ident[:])
                        if po < KT8:
                            nc.vector.tensor_copy(
                                out=wqT8[:, po, r * P:(r + 1) * P],
                                in_=psA[:],
                            )
                        else:
                            nc.vector.tensor_copy(
                                out=wqT16[:, po - KT8, r * P:(r + 1) * P],
                                in_=psA[:],
                            )

            # ---- main matmul loop over m-tiles ----
            for j in range(MT):
                xT = xTp.tile([P, KT, P], BF16, tag="xT", name=f"xT_{j}")
                xread = nc.sync.dma_start_transpose(xT[:], xhat[j][:])
                if j + CAST_AHEAD < MT:
                    add_dep_helper(
                        cast_insts[j + CAST_AHEAD].ins,
                        xread.ins,
                        reason="throttle x-cast to stay a few m-tiles ahead",
                    )
                # fp8 copy of the DoubleRow k-range on the otherwise-idle
                # ACT engine
                xT8 = xT8p.tile([P, KT8, P], FP8, tag="xT8", name=f"xT8_{j}")
                nc.scalar.copy(xT8[:], xT[:, 0:KT8, :])
                psums = [
                    psump.tile([P, NBS], F32, tag="ps", name=f"ps_{j}_{nb}")
                    for nb in range(NB)
                ]

                def mm_fp8(nb, tp, start):
                    nc.tensor.matmul(
                        psums[nb][:],
                        xT8[:, 2 * tp:2 * tp + 2, :],
                        wqT8[:, 2 * tp:2 * tp + 2, nb * NBS:(nb + 1) * NBS],
                        start=start,
                        stop=False,
                        perf_mode=DR,
                    )

                def mm_bf16(nb, kt, start, stop):
                    nc.tensor.matmul(
                        psums[nb][:],
                        xT[:, kt, :],
                        wqT16[:, kt - KT8, nb * NBS:(nb + 1) * NBS],
                        start=start,
                        stop=stop,
                    )

                if j < RAMP_TILES:
                    # nb-outer during ramp: each accumulation gates on only a
                    # quarter of the weight tiles, so matmuls start before the
                    # weight prologue finishes
                    for nb in range(NB):
                        for tp in range(PA):
                            mm_fp8(nb, tp, start=(tp == 0))
                        for kt in range(KT8, KT):
                            mm_bf16(nb, kt, start=(kt == 0), stop=(kt == KT - 1))
                else:
                    for tp in range(PA):
                        for nb in range(NB):
                            mm_fp8(nb, tp, start=(tp == 0))
                    for kt in range(KT8, KT):
                        for nb in range(NB):
                            mm_bf16(nb, kt, start=(kt == 0), stop=(kt == KT - 1))
                osb = osbp.tile([P, NS], F32, tag="osb", name=f"osb_{j}")
                for nb in range(NB):
                    nc.vector.tensor_scalar(
                        osb[:, nb * NBS:(nb + 1) * NBS],
                        psums[nb][:],
                        scal[:, 2:3],
                        None,
                        mybir.AluOpType.mult,
                    )
                nc.vector.tensor_tensor(
                    osb[:], osb[:], bias_sb[:], mybir.AluOpType.add
                )
                # output stores on the second HWDGE queue (Scalar), off the
                # transpose-only Sync queue
                nc.scalar.dma_start(out=y_out[j * P:(j + 1) * P, :], in_=osb[:])

    nc.compile()
    return nc


def _compute_gamma(weight: np.ndarray) -> np.float32:
    """Replicate the module's gamma computation bit-exactly (jnp, fp32)."""
    import jax
    import jax.numpy as jnp

    with jax.default_device(jax.devices("cpu")[0]):
        w_f32 = jnp.clip(jnp.asarray(weight, dtype=jnp.float32), -2.0, 2.0)
        gamma = jnp.maximum(jnp.mean(jnp.abs(w_f32)), 1e-4)
        return np.float32(np.asarray(gamma))


def kernel(x: np.ndarray, weight: np.ndarray, bias: np.ndarray) -> np.ndarray:
    global _NC_CACHE, LAST_RESULTS

    x2d = np.ascontiguousarray(np.asarray(x, dtype=np.float32).reshape(M, K))
    weight = np.ascontiguousarray(np.asarray(weight, dtype=np.float32))
    bias = np.asarray(bias, dtype=np.float32)

    gamma = _compute_gamma(weight)
    thr = np.float32(np.float32(0.5) * gamma)
    scal = np.zeros((P, 4), dtype=np.float32)
    scal[:, 0] = thr
    scal[:, 1] = -thr
    scal[:, 2] = np.float32(np.float32(0.5) * gamma)  # psum carries 2x ternary

    if _NC_CACHE is None:
        _NC_CACHE = _build_nc()
    nc = _NC_CACHE

    in_maps = []
    for i in range(N_CORES):
        w_shard = np.ascontiguousarray(weight[i * NS:(i + 1) * NS])
        b_shard = np.ascontiguousarray(
            np.broadcast_to(bias[i * NS:(i + 1) * NS], (P, NS))
        )
        in_maps.append({"x": x2d, "w": w_shard, "bias": b_shard, "scal": scal})

    res = run_bass_kernel_spmd(nc, in_maps, list(range(N_CORES)))
    LAST_RESULTS = res

    out = np.concatenate([res.results[i]["out"] for i in range(N_CORES)], axis=1)
    return np.ascontiguousarray(out.reshape(B, S, D_OUT))


# revision 12
# speedup vs baseline: 1.0970x; 1.0970x over previous
"""BitLinear (ternary-quantized linear) Trainium2 kernel.

out = x @ (gamma * ternary(weight)).T + bias, computed tensor-parallel over
8 NeuronCores: weight/bias sharded along out_features, x replicated.

Per-core device program (hybrid fp8-DoubleRow / bf16 matmul):
  1. Cast x (fp32) -> bf16 into DRAM scratch via SWDGE casting DMAs,
     throttled to stay a few m-tiles ahead of consumption.
  2. Quantize the weight shard to doubled ternary {-2,0,2} on the ACT
     engine: q' = sign(w - thr) + sign(w + thr) with thr = 0.5*gamma,
     equivalent to 2*clip(round(w/gamma), -1, 1) for all inputs; the factor
     2 is folded into the output scale (gamma/2, exact in fp32).
  3. PE-transpose q' into SBUF-resident [K-partition, k-subtile, N] weight
     tiles: the first KT8 k-subtiles in fp8e4 (exact for {-2,0,2}), the
     remaining KT16 subtiles in bf16. XBAR DMA-transpose x_bf16 tiles into
     [K-partition] layout; ACT-cast the first KT8 subtiles to fp8e4.
  4. Matmuls accumulate fp32 in PSUM: the fp8 k-range runs as DoubleRow
     matmuls (2 k-subtiles / 256 contraction rows per instruction, 2x PE
     throughput — measured streaming at the same ~219 ns per 512-wide
     matmul as bf16), the bf16 k-range as regular 128x128x512 matmuls.
     The fp8 path quantizes x to e4m3 (~2.7e-2 per-element rel RMS);
     keeping KT16 subtiles in bf16 scales the output error by
     ~sqrt(KT8/32), keeping it under the accuracy gate (measured
     1.64e-2 at KT8=16, 1.7-1.8e-2 expected at KT8=18, gate 2e-2).
  5. Drain: psum * (gamma/2) + bias on DVE, DMA out on the Scalar queue.

gamma = max(mean(|clip(w, -2, 2)|), 1e-4) is a global scalar over the full
weight; it is computed on host with the same jnp ops the module uses so the
quantization boundary matches bit-exactly, and enters the device kernel as a
[128, 4] scalar input tensor (threshold, -threshold, gamma).
"""

import numpy as np

import concourse.bass as bass
import concourse.mybir as mybir
import concourse.tile as tile
from concourse import bacc
from concourse.bass_utils import run_bass_kernel_spmd
from concourse.tile import add_dep_helper
from concourse.masks import make_identity

P = 128
B, S, D_IN, D_OUT = 4, 2048, 4096, 16384
M = B * S                 # 8192 tokens
K = D_IN                  # 4096 contraction
N_CORES = 8
NS = D_OUT // N_CORES     # 2048 out-features per core
KT = K // P               # 32 k-subtiles
MT = M // P               # 64 m-tiles
NBS = 512                 # psum bank free size (fp32)
NB = NS // NBS            # 4 psum n-blocks
QCH = 1024                # weight-quantize chunk free size

PA = 9                    # fp8 DoubleRow k-subtile PAIRS (2*PA subtiles fp8)
KT8 = 2 * PA              # k-subtiles on the fp8 path
KT16 = KT - KT8           # k-subtiles on the bf16 path

F32 = mybir.dt.float32
BF16 = mybir.dt.bfloat16
FP8 = mybir.dt.float8e4
DR = mybir.MatmulPerfMode.DoubleRow

_NC_CACHE = None
LAST_RESULTS = None


def _build_nc():
    nc = bacc.Bacc(None, target_bir_lowering=False, debug=False)

    x_in = nc.declare_dram_parameter("x", [M, K], F32, isOutput=False)
    w_in = nc.declare_dram_parameter("w", [NS, K], F32, isOutput=False)
    b_in = nc.declare_dram_parameter("bias", [P, NS], F32, isOutput=False)
    s_in = nc.declare_dram_parameter("scal", [P, 4], F32, isOutput=False)
    y_out = nc.declare_dram_parameter("out", [M, NS], F32, isOutput=True)

    CAST_AHEAD = 2
    RAMP_TILES = 16

    with tile.TileContext(nc) as tc:
        with (
            tc.tile_pool(name="const", bufs=1) as constp,
            tc.tile_pool(name="w_sb", bufs=3) as wsbp,
            tc.tile_pool(name="qab", bufs=6) as qabp,
            tc.tile_pool(name="xT", bufs=2) as xTp,
            tc.tile_pool(name="xT8", bufs=2) as xT8p,
            tc.tile_pool(name="osb", bufs=3) as osbp,
            tc.tile_pool(name="psum", bufs=8, space="PSUM") as psump,
            tc.tile_pool(name="dram", bufs=1, space="DRAM") as dramp,
        ):
            scal = constp.tile([P, 4], F32)
            nc.sync.dma_start(out=scal[:], in_=s_in[:])
            bias_sb = constp.tile([P, NS], F32)
            # full quantized-transposed weight shard, resident in SBUF:
            # fp8 range (DoubleRow path) + bf16 range
            wqT8 = constp.tile([P, KT8, NS], FP8, name="wqT8")
            wqT16 = constp.tile([P, KT16, NS], BF16, name="wqT16")

            # identity for PE transposes: emitted before the cast DMAs so it
            # is not queued behind them on the gpsimd queue
            ident = constp.tile([P, P], BF16)
            make_identity(nc, ident)

            # ---- x fp32 -> bf16 cast, DRAM->DRAM on SWDGE ----
            # Throttled below so the casts stay a few m-tiles ahead of
            # consumption instead of hogging HBM during the prologue.
            xhat = []
            cast_insts = []
            for j in range(MT):
                xh = dramp.tile([P, K], BF16, name=f"xhat_{j}")
                ci = nc.gpsimd.dma_start(out=xh[:], in_=x_in[j * P:(j + 1) * P, :])
                xhat.append(xh)
                cast_insts.append(ci)

            # ---- weight shard: quantize to doubled-ternary, transpose ----
            # q' = sign(w - thr) + sign(w + thr) in {-2, 0, 2}; the factor 2
            # is folded into the output scale (gamma/2). Signs run on the
            # otherwise-idle ACT engine, transposes on the PE, the add and
            # psum-evict on DVE. Subtiles < KT8 land in wqT8 as fp8e4
            # ({-2,0,2} is exact), the rest in wqT16 as bf16.
            for r in range(NS // P):
                for c in range(K // QCH):
                    w_sb = wsbp.tile([P, QCH], F32, tag="w_in")
                    # weight chunks alternate between the Sync and Vector
                    # DMA queues: a single queue carrying all 33.5MB paces
                    # the quantize prologue, and parking them all on the
                    # Scalar queue stalls the ACT sequencer between signs
                    wq_eng = nc.sync if (r * 4 + c) % 2 == 0 else nc.vector
                    wq_eng.dma_start(
                        out=w_sb[:],
                        in_=w_in[r * P:(r + 1) * P, c * QCH:(c + 1) * QCH],
                    )
                    sa = qabp.tile([P, QCH], BF16, tag="q")
                    sb = qabp.tile([P, QCH], BF16, tag="q")
                    nc.scalar.sign(sa[:], w_sb[:], bias=scal[:, 1:2])  # -thr
                    nc.scalar.sign(sb[:], w_sb[:], bias=scal[:, 0:1])  # +thr
                    nc.vector.tensor_tensor(
                        sa[:], sa[:], sb[:], mybir.AluOpType.add
                    )
                    for kk in range(QCH // P):
                        po = c * (QCH // P) + kk
                        psA = psump.tile([P, P], BF16, tag="ps", name=f"tp_{r}_{po}")
                        nc.tensor.transpose(psA[:], sa[:, kk * P:(kk + 1) * P], ident[:])
                        if po < KT8:
                            nc.vector.tensor_copy(
                                out=wqT8[:, po, r * P:(r + 1) * P],
                                in_=psA[:],
                            )
                        else:
                            nc.vector.tensor_copy(
                                out=wqT16[:, po - KT8, r * P:(r + 1) * P],
                                in_=psA[:],
                            )

            # bias load deferred here: it is only needed at the first
            # eviction, and putting its 1MB ahead of the weight chunks on
            # the Sync queue would delay the quantize chain
            nc.sync.dma_start(out=bias_sb[:], in_=b_in[:])

            # ---- main matmul loop over m-tiles ----
            for j in range(MT):
                xT = xTp.tile([P, KT, P], BF16, tag="xT", name=f"xT_{j}")
                xread = nc.sync.dma_start_transpose(xT[:], xhat[j][:])
                if j + CAST_AHEAD < MT:
                    add_dep_helper(
                        cast_insts[j + CAST_AHEAD].ins,
                        xread.ins,
                        reason="throttle x-cast to stay a few m-tiles ahead",
                    )
                # fp8 copy of the DoubleRow k-range on the otherwise-idle
                # ACT engine
                xT8 = xT8p.tile([P, KT8, P], FP8, tag="xT8", name=f"xT8_{j}")
                nc.scalar.copy(xT8[:], xT[:, 0:KT8, :])
                psums = [
                    psump.tile([P, NBS], F32, tag="ps", name=f"ps_{j}_{nb}")
                    for nb in range(NB)
                ]

                def mm_fp8(nb, tp, start):
                    nc.tensor.matmul(
                        psums[nb][:],
                        xT8[:, 2 * tp:2 * tp + 2, :],
                        wqT8[:, 2 * tp:2 * tp + 2, nb * NBS:(nb + 1) * NBS],
                        start=start,
                        stop=False,
                        perf_mode=DR,
                    )

                def mm_bf16(nb, kt, start, stop):
                    nc.tensor.matmul(
                        psums[nb][:],
                        xT[:, kt, :],
                        wqT16[:, kt - KT8, nb * NBS:(nb + 1) * NBS],
                        start=start,
                        stop=stop,
                    )

                if j < RAMP_TILES:
                    # nb-outer during ramp: each accumulation gates on only a
                    # quarter of the weight tiles, so matmuls start before the
                    # weight prologue finishes
                    for nb in range(NB):
                        for tp in range(PA):
                            mm_fp8(nb, tp, start=(tp == 0))
                        for kt in range(KT8, KT):
                            mm_bf16(nb, kt, start=(kt == 0), stop=(kt == KT - 1))
                else:
                    for tp in range(PA):
                        for nb in range(NB):
                            mm_fp8(nb, tp, start=(tp == 0))
                    for kt in range(KT8, KT):
                        for nb in range(NB):
                            mm_bf16(nb, kt, start=(kt == 0), stop=(kt == KT - 1))
                osb = osbp.tile([P, NS], F32, tag="osb", name=f"osb_{j}")
                for nb in range(NB):
                    nc.vector.tensor_scalar(
                        osb[:, nb * NBS:(nb + 1) * NBS],
                        psums[nb][:],
                        scal[:, 2:3],
                        None,
                        mybir.AluOpType.mult,
                    )
                nc.vector.tensor_tensor(
                    osb[:], osb[:], bias_sb[:], mybir.AluOpType.add
                )
                # output stores on the second HWDGE queue (Scalar), off the
                # transpose-only Sync queue
                nc.scalar.dma_start(out=y_out[j * P:(j + 1) * P, :], in_=osb[:])

    nc.compile()
    return nc


def _compute_gamma(weight: np.ndarray) -> np.float32:
    """Replicate the module's gamma computation bit-exactly (jnp, fp32)."""
    import jax
    import jax.numpy as jnp

    with jax.default_device(jax.devices("cpu")[0]):
        w_f32 = jnp.clip(jnp.asarray(weight, dtype=jnp.float32), -2.0, 2.0)
        gamma = jnp.maximum(jnp.mean(jnp.abs(w_f32)), 1e-4)
        return np.float32(np.asarray(gamma))


def kernel(x: np.ndarray, weight: np.ndarray, bias: np.ndarray) -> np.ndarray:
    global _NC_CACHE, LAST_RESULTS

    x2d = np.ascontiguousarray(np.asarray(x, dtype=np.float32).reshape(M, K))
    weight = np.ascontiguousarray(np.asarray(weight, dtype=np.float32))
    bias = np.asarray(bias, dtype=np.float32)

    gamma = _compute_gamma(weight)
    thr = np.float32(np.float32(0.5) * gamma)
    scal = np.zeros((P, 4), dtype=np.float32)
    scal[:, 0] = thr
    scal[:, 1] = -thr
    scal[:, 2] = np.float32(np.float32(0.5) * gamma)  # psum carries 2x ternary

    if _NC_CACHE is None:
        _NC_CACHE = _build_nc()
    nc = _NC_CACHE

    in_maps = []
    for i in range(N_CORES):
        w_shard = np.ascontiguousarray(weight[i * NS:(i + 1) * NS])
        b_shard = np.ascontiguousarray(
            np.broadcast_to(bias[i * NS:(i + 1) * NS], (P, NS))
        )
        in_maps.append({"x": x2d, "w": w_shard, "bias": b_shard, "scal": scal})

    res = run_bass_kernel_spmd(nc, in_maps, list(range(N_CORES)))
    LAST_RESULTS = res

    out = np.concatenate([res.results[i]["out"] for i in range(N_CORES)], axis=1)
    return np.ascontiguousarray(out.reshape(B, S, D_OUT))


# revision 13
# speedup vs baseline: 1.1529x; 1.0510x over previous
"""BitLinear (ternary-quantized linear) Trainium2 kernel.

out = x @ (gamma * ternary(weight)).T + bias, computed tensor-parallel over
8 NeuronCores: weight/bias sharded along out_features, x replicated.

Per-core device program (hybrid fp8-DoubleRow / bf16 matmul):
  1. Cast x (fp32) -> bf16 into DRAM scratch via SWDGE casting DMAs,
     throttled to stay a few m-tiles ahead of consumption.
  2. Quantize the weight shard to doubled ternary {-2,0,2} on the ACT
     engine: q' = sign(w - thr) + sign(w + thr) with thr = 0.5*gamma,
     equivalent to 2*clip(round(w/gamma), -1, 1) for all inputs; the factor
     2 is folded into the output scale (gamma/2, exact in fp32).
  3. PE-transpose q' into SBUF-resident [K-partition, k-subtile, N] weight
     tiles: the first KT8 k-subtiles in fp8e4 (exact for {-2,0,2}), the
     remaining KT16 subtiles in bf16. XBAR DMA-transpose x_bf16 tiles into
     [K-partition] layout; ACT-cast the first KT8 subtiles to fp8e4.
  4. Matmuls accumulate fp32 in PSUM: the fp8 k-range runs as DoubleRow
     matmuls (2 k-subtiles / 256 contraction rows per instruction, 2x PE
     throughput — measured streaming at the same ~219 ns per 512-wide
     matmul as bf16), the bf16 k-range as regular 128x128x512 matmuls.
     The fp8 path quantizes x to e4m3 (~2.7e-2 per-element rel RMS);
     keeping KT16 subtiles in bf16 scales the output error by
     ~sqrt(KT8/32), keeping it under the accuracy gate (measured
     1.64e-2 at KT8=16, 1.74e-2 at KT8=18, ~1.83e-2 at KT8=20; gate 2e-2).
  5. Drain: psum * (gamma/2) + bias on DVE, DMA out on the Scalar queue.

gamma = max(mean(|clip(w, -2, 2)|), 1e-4) is a global scalar over the full
weight; it is computed on host with the same jnp ops the module uses so the
quantization boundary matches bit-exactly, and enters the device kernel as a
[128, 4] scalar input tensor (threshold, -threshold, gamma).
"""

import numpy as np

import concourse.bass as bass
import concourse.mybir as mybir
import concourse.tile as tile
from concourse import bacc
from concourse.bass_utils import run_bass_kernel_spmd
from concourse.tile import add_dep_helper
from concourse.masks import make_identity

P = 128
B, S, D_IN, D_OUT = 4, 2048, 4096, 16384
M = B * S                 # 8192 tokens
K = D_IN                  # 4096 contraction
N_CORES = 8
NS = D_OUT // N_CORES     # 2048 out-features per core
KT = K // P               # 32 k-subtiles
MT = M // P               # 64 m-tiles
NBS = 512                 # psum bank free size (fp32)
NB = NS // NBS            # 4 psum n-blocks
QCH = 1024                # weight-quantize chunk free size

PA = 10                   # fp8 DoubleRow k-subtile PAIRS (2*PA subtiles fp8)
KT8 = 2 * PA              # k-subtiles on the fp8 path
KT16 = KT - KT8           # k-subtiles on the bf16 path

F32 = mybir.dt.float32
BF16 = mybir.dt.bfloat16
FP8 = mybir.dt.float8e4
DR = mybir.MatmulPerfMode.DoubleRow

_NC_CACHE = None
LAST_RESULTS = None


def _build_nc():
    nc = bacc.Bacc(None, target_bir_lowering=False, debug=False)

    x_in = nc.declare_dram_parameter("x", [M, K], F32, isOutput=False)
    w_in = nc.declare_dram_parameter("w", [NS, K], F32, isOutput=False)
    b_in = nc.declare_dram_parameter("bias", [P, NS], F32, isOutput=False)
    s_in = nc.declare_dram_parameter("scal", [P, 4], F32, isOutput=False)
    y_out = nc.declare_dram_parameter("out", [M, NS], F32, isOutput=True)

    CAST_AHEAD = 2
    RAMP_TILES = 16

    with tile.TileContext(nc) as tc:
        with (
            tc.tile_pool(name="const", bufs=1) as constp,
            tc.tile_pool(name="w_sb", bufs=3) as wsbp,
            tc.tile_pool(name="qab", bufs=6) as qabp,
            tc.tile_pool(name="xT", bufs=2) as xTp,
            tc.tile_pool(name="xT8", bufs=2) as xT8p,
            tc.tile_pool(name="osb", bufs=3) as osbp,
            tc.tile_pool(name="psum", bufs=8, space="PSUM") as psump,
            tc.tile_pool(name="dram", bufs=1, space="DRAM") as dramp,
        ):
            scal = constp.tile([P, 4], F32)
            nc.sync.dma_start(out=scal[:], in_=s_in[:])
            bias_sb = constp.tile([P, NS], F32)
            # full quantized-transposed weight shard, resident in SBUF:
            # fp8 range (DoubleRow path) + bf16 range
            wqT8 = constp.tile([P, KT8, NS], FP8, name="wqT8")
            wqT16 = constp.tile([P, KT16, NS], BF16, name="wqT16")

            # identity for PE transposes: emitted before the cast DMAs so it
            # is not queued behind them on the gpsimd queue
            ident = constp.tile([P, P], BF16)
            make_identity(nc, ident)

            # ---- x fp32 -> bf16 cast, DRAM->DRAM on SWDGE ----
            # Throttled below so the casts stay a few m-tiles ahead of
            # consumption instead of hogging HBM during the prologue.
            xhat = []
            cast_insts = []
            for j in range(MT):
                xh = dramp.tile([P, K], BF16, name=f"xhat_{j}")
                ci = nc.gpsimd.dma_start(out=xh[:], in_=x_in[j * P:(j + 1) * P, :])
                xhat.append(xh)
                cast_insts.append(ci)

            # ---- weight shard: quantize to doubled-ternary, transpose ----
            # q' = sign(w - thr) + sign(w + thr) in {-2, 0, 2}; the factor 2
            # is folded into the output scale (gamma/2). Signs run on the
            # otherwise-idle ACT engine, transposes on the PE, the add and
            # psum-evict on DVE. Subtiles < KT8 land in wqT8 as fp8e4
            # ({-2,0,2} is exact), the rest in wqT16 as bf16.
            for r in range(NS // P):
                for c in range(K // QCH):
                    w_sb = wsbp.tile([P, QCH], F32, tag="w_in")
                    # weight chunks alternate between the Sync and Vector
                    # DMA queues: a single queue carrying all 33.5MB paces
                    # the quantize prologue, and parking them all on the
                    # Scalar queue stalls the ACT sequencer between signs
                    wq_eng = nc.sync if (r * 4 + c) % 2 == 0 else nc.vector
                    wq_eng.dma_start(
                        out=w_sb[:],
                        in_=w_in[r * P:(r + 1) * P, c * QCH:(c + 1) * QCH],
                    )
                    sa = qabp.tile([P, QCH], BF16, tag="q")
                    sb = qabp.tile([P, QCH], BF16, tag="q")
                    nc.scalar.sign(sa[:], w_sb[:], bias=scal[:, 1:2])  # -thr
                    nc.scalar.sign(sb[:], w_sb[:], bias=scal[:, 0:1])  # +thr
                    nc.vector.tensor_tensor(
                        sa[:], sa[:], sb[:], mybir.AluOpType.add
                    )
                    for kk in range(QCH // P):
                        po = c * (QCH // P) + kk
                        psA = psump.tile([P, P], BF16, tag="ps", name=f"tp_{r}_{po}")
                        nc.tensor.transpose(psA[:], sa[:, kk * P:(kk + 1) * P], ident[:])
                        if po < KT8:
                            nc.vector.tensor_copy(
                                out=wqT8[:, po, r * P:(r + 1) * P],
                                in_=psA[:],
                            )
                        else:
                            nc.vector.tensor_copy(
                                out=wqT16[:, po - KT8, r * P:(r + 1) * P],
                                in_=psA[:],
                            )

            # bias load deferred here: it is only needed at the first
            # eviction, and putting its 1MB ahead of the weight chunks on
            # the Sync queue would delay the quantize chain
            nc.sync.dma_start(out=bias_sb[:], in_=b_in[:])

            # ---- main matmul loop over m-tiles ----
            for j in range(MT):
                xT = xTp.tile([P, KT, P], BF16, tag="xT", name=f"xT_{j}")
                xread = nc.sync.dma_start_transpose(xT[:], xhat[j][:])
                if j + CAST_AHEAD < MT:
                    add_dep_helper(
                        cast_insts[j + CAST_AHEAD].ins,
                        xread.ins,
                        reason="throttle x-cast to stay a few m-tiles ahead",
                    )
                # fp8 copy of the DoubleRow k-range on the otherwise-idle
                # ACT engine
                xT8 = xT8p.tile([P, KT8, P], FP8, tag="xT8", name=f"xT8_{j}")
                nc.scalar.copy(xT8[:], xT[:, 0:KT8, :])
                psums = [
                    psump.tile([P, NBS], F32, tag="ps", name=f"ps_{j}_{nb}")
                    for nb in range(NB)
                ]

                def mm_fp8(nb, tp, start):
                    nc.tensor.matmul(
                        psums[nb][:],
                        xT8[:, 2 * tp:2 * tp + 2, :],
                        wqT8[:, 2 * tp:2 * tp + 2, nb * NBS:(nb + 1) * NBS],
                        start=start,
                        stop=False,
                        perf_mode=DR,
                    )

                def mm_bf16(nb, kt, start, stop):
                    nc.tensor.matmul(
                        psums[nb][:],
                        xT[:, kt, :],
                        wqT16[:, kt - KT8, nb * NBS:(nb + 1) * NBS],
                        start=start,
                        stop=stop,
                    )

                if j < RAMP_TILES:
                    # nb-outer during ramp: each accumulation gates on only a
                    # quarter of the weight tiles, so matmuls start before the
                    # weight prologue finishes
                    for nb in range(NB):
                        for tp in range(PA):
                            mm_fp8(nb, tp, start=(tp == 0))
                        for kt in range(KT8, KT):
                            mm_bf16(nb, kt, start=(kt == 0), stop=(kt == KT - 1))
                else:
                    for tp in range(PA):
                        for nb in range(NB):
                            mm_fp8(nb, tp, start=(tp == 0))
                    for kt in range(KT8, KT):
                        for nb in range(NB):
                            mm_bf16(nb, kt, start=(kt == 0), stop=(kt == KT - 1))
                osb = osbp.tile([P, NS], F32, tag="osb", name=f"osb_{j}")
                for nb in range(NB):
                    nc.vector.tensor_scalar(
                        osb[:, nb * NBS:(nb + 1) * NBS],
                        psums[nb][:],
                        scal[:, 2:3],
                        None,
                        mybir.AluOpType.mult,
                    )
                nc.vector.tensor_tensor(
                    osb[:], osb[:], bias_sb[:], mybir.AluOpType.add
                )
                # output stores on the second HWDGE queue (Scalar), off the
                # transpose-only Sync queue
                nc.scalar.dma_start(out=y_out[j * P:(j + 1) * P, :], in_=osb[:])

    nc.compile()
    return nc


def _compute_gamma(weight: np.ndarray) -> np.float32:
    """Replicate the module's gamma computation bit-exactly (jnp, fp32)."""
    import jax
    import jax.numpy as jnp

    with jax.default_device(jax.devices("cpu")[0]):
        w_f32 = jnp.clip(jnp.asarray(weight, dtype=jnp.float32), -2.0, 2.0)
        gamma = jnp.maximum(jnp.mean(jnp.abs(w_f32)), 1e-4)
        return np.float32(np.asarray(gamma))


def kernel(x: np.ndarray, weight: np.ndarray, bias: np.ndarray) -> np.ndarray:
    global _NC_CACHE, LAST_RESULTS

    x2d = np.ascontiguousarray(np.asarray(x, dtype=np.float32).reshape(M, K))
    weight = np.ascontiguousarray(np.asarray(weight, dtype=np.float32))
    bias = np.asarray(bias, dtype=np.float32)

    gamma = _compute_gamma(weight)
    thr = np.float32(np.float32(0.5) * gamma)
    scal = np.zeros((P, 4), dtype=np.float32)
    scal[:, 0] = thr
    scal[:, 1] = -thr
    scal[:, 2] = np.float32(np.float32(0.5) * gamma)  # psum carries 2x ternary

    if _NC_CACHE is None:
        _NC_CACHE = _build_nc()
    nc = _NC_CACHE

    in_maps = []
    for i in range(N_CORES):
        w_shard = np.ascontiguousarray(weight[i * NS:(i + 1) * NS])
        b_shard = np.ascontiguousarray(
            np.broadcast_to(bias[i * NS:(i + 1) * NS], (P, NS))
        )
        in_maps.append({"x": x2d, "w": w_shard, "bias": b_shard, "scal": scal})

    res = run_bass_kernel_spmd(nc, in_maps, list(range(N_CORES)))
    LAST_RESULTS = res

    out = np.concatenate([res.results[i]["out"] for i in range(N_CORES)], axis=1)
    return np.ascontiguousarray(out.reshape(B, S, D_OUT))
